# revision 1
# baseline (speedup 1.0000x reference)
"""Trainium2 Bass kernel for nn_Attention_35854386987485.

Math (per batch row b):
    hp   = h @ W_h                               (bias folded later)
    vp   = v[b,t] @ W_v
    z3   = tanh(vp + hp + (b_v + b_h))           [T, H]
    z    = z3 @ w_z + b_z                        [T]
    beta = tanh((s @ W_s + hp + (b_s+b_h)) * sqrt(.5)) @ w_beta + b_beta
    a    = softmax([z, beta])                    [T+1]
    c    = sum_t a_t * [v; s][t]                 [H]

Data-parallel over batch across 8 NeuronCores; each core processes B=512 rows.

Per-core dataflow:
  * v is cast-loaded (fp32->bf16, SWDGE) in a "v2" layout [t + 64*par (p), slot, h]
    with s injected at t-row 49  (8 batch rows per chunk, b = 2*slot+par).
  * One xbar DMA-transpose per chunk turns v2 into vT [h%128 (p), (slot,ht,64par+t)]
    which feeds the main W_v matmul (bf16, moving operand, packed 392-col AP).
  * hp is broadcast-added over t on DVE, tanh on ACT (bias = b_v+b_h per partition),
    z-reduction via PE matvec with w_z columns, softmax per 32-row group,
    attention-apply (op2) via tile-packed PE matvecs on the v2 layout.
"""

import os
import sys
from contextlib import ExitStack

sys.path.insert(0, "/opt/trn_rl_repo")

import numpy as np

import concourse.bass as bass
import concourse.bacc as bacc
import concourse.tile as tile
from concourse import masks, mybir

F32 = mybir.dt.float32
BF16 = mybir.dt.bfloat16
AF = mybir.ActivationFunctionType
ALU = mybir.AluOpType
AX = mybir.AxisListType

T = 49
H = 512
NB = 8          # batch rows per chunk
TP = 64         # padded t rows per parity in the v2 layout (row 49 = s)
NCOL = NB * T   # packed (slot,par,t) columns per chunk = 392
SQ5 = float(np.sqrt(0.5))

N_CORES = 8
B_TOTAL = 4096


def build_bass(B):
    """Build the per-core Bass program for per-core batch size B (mult of 32)."""
    assert B % 32 == 0
    NCH = B // NB          # chunks
    NGRP = NCH // 4        # softmax groups (32 rows each)
    P0 = min(B, 128)       # h/s natural-tile partition count
    NBT = max(B // 128, 1)  # 128-row tiles of h/s
    assert B <= 128 or B % 128 == 0

    nc = bacc.Bacc("TRN2", target_bir_lowering=False, debug=False,
                   num_devices=N_CORES)

    v = nc.dram_tensor("v", (B, T, H), F32, kind="ExternalInput").ap()
    hh = nc.dram_tensor("h", (B, H), F32, kind="ExternalInput").ap()
    ss = nc.dram_tensor("s", (B, H), F32, kind="ExternalInput").ap()
    W_h = nc.dram_tensor("W_h", (H, H), F32, kind="ExternalInput").ap()
    b_h = nc.dram_tensor("b_h", (H,), F32, kind="ExternalInput").ap()
    W_v = nc.dram_tensor("W_v", (H, H), F32, kind="ExternalInput").ap()
    b_v = nc.dram_tensor("b_v", (H,), F32, kind="ExternalInput").ap()
    w_z = nc.dram_tensor("w_z", (H,), F32, kind="ExternalInput").ap()
    b_z = nc.dram_tensor("b_z", (1,), F32, kind="ExternalInput").ap()
    W_s = nc.dram_tensor("W_s", (H, H), F32, kind="ExternalInput").ap()
    b_s = nc.dram_tensor("b_s", (H,), F32, kind="ExternalInput").ap()
    w_beta = nc.dram_tensor("w_beta", (H,), F32, kind="ExternalInput").ap()
    b_beta = nc.dram_tensor("b_beta", (1,), F32, kind="ExternalInput").ap()
    c = nc.dram_tensor("c", (B, H), F32, kind="ExternalOutput").ap()
    zl = nc.dram_tensor("zl_scratch", (B, T + 1), F32, kind="Internal").ap()

    with tile.TileContext(nc) as tc, ExitStack() as ctx:
        consts = ctx.enter_context(tc.tile_pool(name="consts", bufs=1))

        # --- constant loads (HWDGE fp32, engine-side bf16 casts) ------------
        wv = consts.tile([128, 4, H], BF16)          # W_v[ki*128+p, ho]
        wh = consts.tile([128, 4, H], BF16)
        wst = consts.tile([128, 4, H], BF16)
        wzc = consts.tile([128, 4], BF16)            # w_z[ht*128+p]
        wbc = consts.tile([128, 4], BF16)            # w_beta[ht*128+p]
        with ExitStack() as cpre:
            cstg = cpre.enter_context(tc.tile_pool(name="cstg", bufs=2))
            for src, dst in ((W_v, wv), (W_h, wh), (W_s, wst)):
                stg = cstg.tile([128, 4, H], F32, tag="wstg")
                nc.sync.dma_start(stg[:], src.rearrange("(ki p) ho -> p ki ho",
                                                        p=128))
                nc.any.tensor_copy(dst[:], stg[:])
            for src, dst in ((w_z, wzc), (w_beta, wbc)):
                stg = cstg.tile([128, 4], F32, tag="vstg")
                nc.sync.dma_start(stg[:], src.rearrange("(ht p) -> p ht", p=128))
                nc.any.tensor_copy(dst[:], stg[:])

        bvt = consts.tile([128, 4], F32)
        nc.sync.dma_start(bvt[:], b_v.rearrange("(ht p) -> p ht", p=128))
        bht = consts.tile([128, 4], F32)
        nc.sync.dma_start(bht[:], b_h.rearrange("(ht p) -> p ht", p=128))
        bst = consts.tile([128, 4], F32)
        nc.sync.dma_start(bst[:], b_s.rearrange("(ht p) -> p ht", p=128))
        bvh = consts.tile([128, 4], F32)             # b_v + b_h
        nc.vector.tensor_add(bvh[:], bvt[:], bht[:])
        bsb = consts.tile([128, 4], F32)             # sqrt(.5) * (b_s + b_h)
        nc.vector.tensor_add(bsb[:], bst[:], bht[:])
        nc.scalar.mul(bsb[:], bsb[:], SQ5)

        # softmax logits are shift-invariant: [z+b_z, beta+b_beta] ~ [z, beta+(b_beta-b_z)]
        bzt = consts.tile([1, 1], F32)
        nc.sync.dma_start(bzt[:], b_z.unsqueeze(0))
        bbr = consts.tile([1, 1], F32)               # b_beta - b_z
        nc.sync.dma_start(bbr[:], b_beta.unsqueeze(0))
        nc.vector.tensor_sub(bbr[:], bbr[:], bzt[:])

        identb = consts.tile([128, 128], BF16)
        masks.make_identity(nc, identb[:])
        identf = consts.tile([128, 128], F32)
        masks.make_identity(nc, identf[:])

        hpT = consts.tile([128, 4, B], F32)          # (h @ W_h)^T
        beta_row = consts.tile([1, B], F32)          # beta logits

        # --- preamble: hT, sT, hp, beta ------------------------------------
        with ExitStack() as pre:
            prep = pre.enter_context(tc.tile_pool(name="prep", bufs=2))
            pps = pre.enter_context(tc.tile_pool(name="pps", bufs=2, space="PSUM"))

            hT = prep.tile([128, 4, B], BF16, tag="hT")
            sT = prep.tile([128, 4, B], BF16, tag="hT")
            for src, dst in ((hh, hT), (ss, sT)):
                nat = prep.tile([128, NBT, H], F32, tag="nat")
                nc.sync.dma_start(
                    nat[0:P0, :, :], src.rearrange("(bt p) hx -> p bt hx", p=P0))
                for bt in range(NBT):
                    for ht in range(4):
                        pst = pps.tile([128, 512], F32, tag="tp")
                        nc.tensor.transpose(
                            pst[:, 0:P0], nat[0:P0, bt, ht * 128:(ht + 1) * 128],
                            identf[0:P0, 0:P0])
                        nc.vector.tensor_copy(
                            dst[:, ht, bt * P0:(bt + 1) * P0], pst[:, 0:P0])

            for ht in range(4):
                ps = pps.tile([128, 512], F32, tag="mm")
                for ki in range(4):
                    nc.tensor.matmul(ps[:, 0:B], wh[:, ki, ht * 128:(ht + 1) * 128],
                                     hT[:, ki, :], start=(ki == 0), stop=(ki == 3))
                nc.vector.tensor_copy(hpT[:, ht, :], ps[:, 0:B])

            betaT = prep.tile([128, 4, B], BF16, tag="betaT")
            for ht in range(4):
                ps = pps.tile([128, 512], F32, tag="mm")
                for ki in range(4):
                    nc.tensor.matmul(ps[:, 0:B], wst[:, ki, ht * 128:(ht + 1) * 128],
                                     sT[:, ki, :], start=(ki == 0), stop=(ki == 3))
                tmp = prep.tile([128, B], F32, tag="btmp")
                nc.vector.tensor_add(tmp[:], ps[:, 0:B], hpT[:, ht, :])
                nc.scalar.activation(betaT[:, ht, :], tmp[:], AF.Tanh,
                                     bias=bsb[:, ht:ht + 1], scale=SQ5)
            psb = pps.tile([128, 512], F32, tag="mmb")
            for ht in range(4):
                nc.tensor.matmul(psb[0:1, 0:B], wbc[:, ht:ht + 1], betaT[:, ht, :],
                                 start=(ht == 0), stop=(ht == 3))
            nc.scalar.activation(beta_row[:], psb[0:1, 0:B], AF.Identity,
                                 bias=bbr[0:1, 0:1])
            # park beta logits in DRAM scratch column 49
            nc.sync.dma_start(zl[:, T:T + 1].rearrange("b o -> o b"),
                              beta_row[:])

        # --- main loop ------------------------------------------------------
        v2f_pool = ctx.enter_context(tc.tile_pool(name="v2f", bufs=1))
        v2_pool = ctx.enter_context(tc.tile_pool(name="v2", bufs=10))

        # pinned fp32 staging tiles (manual rotation): full memset once so the
        # dead pad rows (t 50..63 per parity) are owned/initialized bytes
        v2f0 = v2f_pool.tile([128, 4, H], F32)
        v2f1 = v2f_pool.tile([128, 4, H], F32)
        v2f2 = v2f_pool.tile([128, 4, H], F32)
        v2f3 = v2f_pool.tile([128, 4, H], F32)
        v2f_tiles = [v2f0, v2f1, v2f2, v2f3]
        for t_ in v2f_tiles:
            nc.vector.memset(t_[:], 0.0)
        vt_pool = ctx.enter_context(tc.tile_pool(name="vt", bufs=5))
        z3p_pool = ctx.enter_context(tc.tile_pool(name="z3p", bufs=3))
        z3b_pool = ctx.enter_context(tc.tile_pool(name="z3b", bufs=3))
        sm_pool = ctx.enter_context(tc.tile_pool(name="sm", bufs=2))
        cst_pool = ctx.enter_context(tc.tile_pool(name="cst", bufs=2))
        mm_ps = ctx.enter_context(tc.tile_pool(name="mmps", bufs=4, space="PSUM"))
        z_ps = ctx.enter_context(tc.tile_pool(name="zps", bufs=2, space="PSUM"))
        c_ps = ctx.enter_context(tc.tile_pool(name="cps", bufs=2, space="PSUM"))

        # pinned aT tiles: zeros outside the valid parity row ranges mask the
        # opposite parity in the full-K op2 matmuls
        aT0 = sm_pool.tile([128, 32], BF16)
        aT1 = sm_pool.tile([128, 32], BF16)
        aT_tiles = [aT0, aT1]
        for t_ in aT_tiles:
            nc.vector.memset(t_[:], 0.0)
        # row padded to 640 so DMA AP lowering can't flat-merge partition
        # pairs into a (wrong) within-partition free run
        cst0 = cst_pool.tile([128, 640], F32)
        cst1 = cst_pool.tile([128, 640], F32)
        cst_tiles = [cst0, cst1]
        for t_ in cst_tiles:
            nc.vector.memset(t_[:], 0.0)

        for grp in range(NGRP):
            zps = z_ps.tile([128, 512], F32)
            v2cs = []
            for cj in range(4):
                ci = grp * 4 + cj
                b0 = ci * NB

                # 1. load v chunk fp32 into v2 layout (s at t=49), cast on GPSIMD
                v2f = v2f_tiles[ci % 4]
                v2fv = v2f[:].rearrange("(par tp) slot hx -> par tp slot hx", tp=TP)
                vsrc = v[b0:b0 + NB].rearrange(
                    "(slot par) t hx -> par t slot hx", par=2)
                ssrc = ss[b0:b0 + NB].rearrange(
                    "(slot par) hx -> par slot hx", par=2)
                for par in (0, 1):
                    # 2-D SBUF-side APs (partitions x contiguous row)
                    nc.sync.dma_start(v2fv[par, 0:T], vsrc[par])
                    nc.sync.dma_start(v2fv[par, T:T + 1],
                                      ssrc[par].unsqueeze(0))
                v2c = v2_pool.tile([128, 4, H], BF16)
                v2cs.append(v2c)
                # rotate the fp32->bf16 cast across engines: GPSIMD is ~4x
                # slower than ACT/DVE here, so it only gets every 4th chunk
                if ci % 4 == 0:
                    nc.gpsimd.tensor_copy(v2c[:], v2f[:])
                elif ci % 4 == 2:
                    nc.vector.tensor_copy(v2c[:], v2f[:])
                else:
                    nc.scalar.copy(v2c[:], v2f[:])

                # 2. xbar transpose -> vT chunk [128, (slot,ht), 64*par+t]
                vtc = vt_pool.tile([128, 16, 128], BF16)
                nc.sync.dma_start_transpose(
                    vtc[:], v2c[:].rearrange("p a b -> p (a b)"))

                # 3+4+5. main matmul, hp broadcast add, tanh
                z3b = z3b_pool.tile([128, 4, NCOL], BF16)
                hpb = hpT[:, :, b0:b0 + NB].rearrange(
                    "p ho (slot par) -> p ho slot par", par=2).unsqueeze(4)
                for ho in range(4):
                    ps = mm_ps.tile([128, 512], F32, tag="mm")
                    rhs = vtc[:].rearrange(
                        "p (slot ht) tq -> p ht slot tq", ht=4)[:, :, :, :].rearrange(
                        "p ht slot (par t) -> p ht slot par t", par=2)
                    for ki in range(4):
                        nc.tensor.matmul(
                            ps[:, 0:NCOL].rearrange("p (slot par t) -> p slot par t",
                                                    slot=4, par=2),
                            wv[:, ki, ho * 128:(ho + 1) * 128],
                            rhs[:, ki, :, :, 0:T],
                            start=(ki == 0), stop=(ki == 3))
                    z3p = z3p_pool.tile([128, NCOL], F32)
                    nc.vector.tensor_tensor(
                        z3p[:].rearrange("p (slot par t) -> p slot par t",
                                         slot=4, par=2),
                        ps[:, 0:NCOL].rearrange("p (slot par t) -> p slot par t",
                                                slot=4, par=2),
                        hpb[:, ho].to_broadcast((128, 4, 2, T)),
                        ALU.add)
                    nc.scalar.activation(z3b[:, ho, :], z3p[:], AF.Tanh,
                                         bias=bvh[:, ho:ho + 1])

                # 6. z-reduction into psum row 32*cj
                for ht in range(4):
                    nc.tensor.matmul(zps[32 * cj:32 * cj + 1, 0:NCOL],
                                     wzc[:, ht:ht + 1], z3b[:, ht, :],
                                     start=(ht == 0), stop=(ht == 3),
                                     tile_position=(0, 32 * cj))

            # 7. drain z rows, park in DRAM scratch, reload as [32, 50]
            zst = sm_pool.tile([128, NCOL], F32, tag="zst")
            for cj in range(4):
                if grp % 2 == 0:
                    nc.vector.tensor_copy(zst[32 * cj:32 * cj + 1, :],
                                          zps[32 * cj:32 * cj + 1, 0:NCOL])
                else:
                    nc.scalar.copy(zst[32 * cj:32 * cj + 1, :],
                                   zps[32 * cj:32 * cj + 1, 0:NCOL])
            nc.sync.dma_start(
                zl[grp * 32:(grp + 1) * 32, 0:T].rearrange(
                    "(cj bb) t -> cj bb t", bb=8),
                zst[:].rearrange("(g r) (bb t) -> g r bb t", r=32, bb=8)[:, 0])
            zg = sm_pool.tile([128, 64], F32, tag="zg")
            nc.sync.dma_start(zg[0:32, 0:T + 1], zl[grp * 32:(grp + 1) * 32, :])

            # 8. softmax over 50 logits for 32 rows
            negm = sm_pool.tile([128, 1], F32, tag="negm")
            nc.vector.tensor_reduce(negm[0:32], zg[0:32, 0:T + 1], axis=AX.X,
                                    op=ALU.max, negate=True)
            ea = sm_pool.tile([128, T + 1], F32, tag="ea")
            nc.scalar.activation(ea[0:32, :], zg[0:32, 0:T + 1], AF.Exp,
                                 bias=negm[0:32, 0:1])
            ssum = sm_pool.tile([128, 1], F32, tag="ssum")
            nc.vector.tensor_reduce(ssum[0:32], ea[0:32, :], axis=AX.X,
                                    op=ALU.add)
            rinv = sm_pool.tile([128, 1], F32, tag="rinv")
            nc.vector.reciprocal(rinv[0:32], ssum[0:32])
            ab = sm_pool.tile([128, T + 1], BF16, tag="ab")
            nc.vector.tensor_scalar_mul(ab[0:32, :], ea[0:32, :],
                                        rinv[0:32, 0:1])

            # 9. transpose a -> aT columns (col = row-in-group bg; even bg
            # valid on partitions 0..49, odd bg on 64..113, zeros elsewhere)
            aT = aT_tiles[grp % 2]
            pa0 = mm_ps.tile([128, 1024], BF16, tag="mm")
            nc.tensor.matmul(pa0[0:T + 1, 0:32], ab[0:32, :], identb[0:32, 0:32],
                             is_transpose=True, tile_position=(0, 0))
            nc.vector.tensor_copy(
                aT[0:T + 1, :].rearrange("p (c2 par) -> p c2 par", par=2)[:, :, 0],
                pa0[0:T + 1, 0:32].rearrange("p (c2 par) -> p c2 par", par=2)[:, :, 0])
            pa1 = mm_ps.tile([128, 1024], BF16, tag="mm")
            nc.tensor.matmul(pa1[64:64 + T + 1, 0:32], ab[0:32, :],
                             identb[0:32, 0:32],
                             is_transpose=True, tile_position=(0, 64))
            nc.vector.tensor_copy(
                aT[64:64 + T + 1, :].rearrange(
                    "p (c2 par) -> p c2 par", par=2)[:, :, 1],
                pa1[64:64 + T + 1, 0:32].rearrange(
                    "p (c2 par) -> p c2 par", par=2)[:, :, 1])

            # 10. attention apply (op2): one M=2 matmul per slot (both
            # parities packed via zero-masked aT columns), col-tiled
            for cj in range(4):
                ci = grp * 4 + cj
                b0 = ci * NB
                v2c = v2cs[cj]
                cps = c_ps.tile([128, H], F32)
                for slot in range(4):
                    bg = cj * 8 + 2 * slot
                    nc.tensor.matmul(
                        cps[32 * slot:32 * slot + 2, :],
                        aT[:, bg:bg + 2],
                        v2c[:, slot, :],
                        start=True, stop=True,
                        tile_position=(0, 32 * slot))
                cst = cst_tiles[ci % 2]
                for slot in range(4):
                    if ci % 2 == 0:
                        nc.vector.tensor_copy(
                            cst[32 * slot:32 * slot + 2, 0:H],
                            cps[32 * slot:32 * slot + 2, :])
                    else:
                        nc.scalar.copy(
                            cst[32 * slot:32 * slot + 2, 0:H],
                            cps[32 * slot:32 * slot + 2, :])
                for slot in range(4):
                    nc.scalar.dma_start(
                        c[b0 + 2 * slot:b0 + 2 * slot + 2],
                        cst[32 * slot:32 * slot + 2, 0:H])

    nc.compile()
    return nc


_NC_CACHE = {}

# test harness hooks: set TRACE=True (with an NTFF profile hook registered)
# to capture HW timing; the BassKernelResults of the last run lands in LAST.
TRACE = False
LAST = {}


def _get_nc(B):
    if B not in _NC_CACHE:
        _NC_CACHE[B] = build_bass(B)
    return _NC_CACHE[B]


def kernel(**inputs):
    from concourse.bass_utils import run_bass_kernel_spmd

    v = np.ascontiguousarray(np.asarray(inputs["v"], dtype=np.float32))
    h = np.ascontiguousarray(np.asarray(inputs["h"], dtype=np.float32))
    s = np.ascontiguousarray(np.asarray(inputs["s"], dtype=np.float32))
    B_total = v.shape[0]
    B = B_total // N_CORES
    nc = _get_nc(B)

    shared = {}
    for k in ("W_h", "b_h", "W_v", "b_v", "w_z", "W_s", "b_s", "w_beta"):
        shared[k] = np.ascontiguousarray(np.asarray(inputs[k], dtype=np.float32))
    for k in ("b_z", "b_beta"):
        shared[k] = np.asarray(inputs[k], dtype=np.float32).reshape(1)

    in_maps = []
    for k in range(N_CORES):
        sl = slice(k * B, (k + 1) * B)
        in_maps.append(dict(shared, v=v[sl], h=h[sl], s=s[sl]))

    kwargs = {"trace": True} if TRACE else {}
    res = run_bass_kernel_spmd(nc, in_maps, core_ids=list(range(N_CORES)),
                               **kwargs)
    LAST["res"] = res
    out = np.concatenate([r["c"] for r in res.results], axis=0)
    return out.astype(np.float32)



# revision 20
# speedup vs baseline: 1.0334x; 1.0334x over previous
"""Trainium2 Bass kernel for nn_Attention_35854386987485 (v2).

Math (per batch row b):
    hp   = h @ W_h                               (bias folded later)
    z3   = tanh(v[b,t] @ W_v + hp + (b_v+b_h))   [T, H]
    z    = z3 @ w_z + b_z                        [T]
    beta = tanh((s @ W_s + hp + (b_s+b_h)) * sqrt(.5)) @ w_beta + b_beta
    a    = softmax([z, beta])                    [T+1]
    c    = sum_t a_t * [v; s][t]                 [H]

Data-parallel over batch across 8 NeuronCores; each core processes B=512 rows.

v2 changes vs baseline (trace-driven):
  * v is cast-loaded fp32->bf16 during DMA (SWDGE) straight into the v2
    layout -- no fp32 staging tiles, no engine-side casts.
  * The hp broadcast-add moved off DVE onto PE: a rank-8 "selector"
    matmul (stationary = this chunk's 8 hp rows staged at partitions
    0-7, moving = a constant 0/1 expansion mask) accumulates hp into
    the same PSUM as the W_v matmuls.  ACT then does tanh straight
    from PSUM with the (b_v+b_h) per-partition bias.
  * Main-matmul moving AP streams runs of 50 (t=0..49 incl. the s row)
    instead of 49 -- even element count keeps the bf16 xbus at full
    rate.  N per chunk is 400 with 8 junk columns (t=49) never read.
  * PSUM drains are single full-tile copies (z: 1 DVE copy/group,
    c: 1 DVE bf16 copy/chunk) instead of per-row copies.
  * z logits / c output round-trip in bf16; c is cast to fp32 on host.
"""

import os
import sys
from contextlib import ExitStack

sys.path.insert(0, "/opt/trn_rl_repo")

import numpy as np

import concourse.bass as bass
import concourse.bacc as bacc
import concourse.tile as tile
from concourse import masks, mybir

F32 = mybir.dt.float32
BF16 = mybir.dt.bfloat16
AF = mybir.ActivationFunctionType
ALU = mybir.AluOpType
AX = mybir.AxisListType

T = 49
H = 512
NB = 8           # batch rows per chunk
TP = 64          # padded t rows per parity in the v2 layout (row 49 = s)
TR = 50          # streamed t rows per parity (incl. s row -> even runs)
NCOL = NB * TR   # packed (slot,par,t50) columns per chunk = 400
SQ5 = float(np.sqrt(0.5))

N_CORES = 8
B_TOTAL = 4096
N_V2 = 10        # pinned v2 chunk tiles in flight
USE_SWDGE = int(os.environ.get("USE_SWDGE", "1"))   # bisect flag
USE_SEL = int(os.environ.get("USE_SEL", "1"))       # bisect flag


def build_bass(B):
    """Build the per-core Bass program for per-core batch size B (mult of 32)."""
    assert B % 32 == 0
    NCH = B // NB          # chunks
    NGRP = NCH // 4        # softmax groups (32 rows each)
    P0 = min(B, 128)       # h/s natural-tile partition count
    NBT = max(B // 128, 1)  # 128-row tiles of h/s
    assert B <= 128 or B % 128 == 0

    nc = bacc.Bacc("TRN2", target_bir_lowering=False, debug=False,
                   num_devices=N_CORES)

    v = nc.dram_tensor("v", (B, T, H), F32, kind="ExternalInput").ap()
    hh = nc.dram_tensor("h", (B, H), F32, kind="ExternalInput").ap()
    ss = nc.dram_tensor("s", (B, H), F32, kind="ExternalInput").ap()
    W_h = nc.dram_tensor("W_h", (H, H), F32, kind="ExternalInput").ap()
    b_h = nc.dram_tensor("b_h", (H,), F32, kind="ExternalInput").ap()
    W_v = nc.dram_tensor("W_v", (H, H), F32, kind="ExternalInput").ap()
    b_v = nc.dram_tensor("b_v", (H,), F32, kind="ExternalInput").ap()
    w_z = nc.dram_tensor("w_z", (H,), F32, kind="ExternalInput").ap()
    b_z = nc.dram_tensor("b_z", (1,), F32, kind="ExternalInput").ap()
    W_s = nc.dram_tensor("W_s", (H, H), F32, kind="ExternalInput").ap()
    b_s = nc.dram_tensor("b_s", (H,), F32, kind="ExternalInput").ap()
    w_beta = nc.dram_tensor("w_beta", (H,), F32, kind="ExternalInput").ap()
    b_beta = nc.dram_tensor("b_beta", (1,), F32, kind="ExternalInput").ap()
    c = nc.dram_tensor("c", (B, H), BF16, kind="ExternalOutput").ap()
    zl = nc.dram_tensor("zl_scratch", (B, T + 1), BF16,
                        kind="ExternalOutput" if os.environ.get("DBG_ZL") else "Internal").ap()
    hpd = nc.dram_tensor("hp_scratch", (B, H), BF16, kind="Internal").ap()

    with tile.TileContext(nc) as tc, ExitStack() as ctx:
        consts = ctx.enter_context(tc.tile_pool(name="consts", bufs=1))

        # --- constant loads (HWDGE fp32, engine-side bf16 casts) ------------
        wv = consts.tile([128, 4, H], BF16)          # W_v[ki*128+p, ho]
        wh = consts.tile([128, 4, H], BF16)
        wst = consts.tile([128, 4, H], BF16)
        wzc = consts.tile([128, 4], BF16)            # w_z[ht*128+p]
        wbc = consts.tile([128, 4], BF16)            # w_beta[ht*128+p]
        with ExitStack() as cpre:
            cstg = cpre.enter_context(tc.tile_pool(name="cstg", bufs=2))
            for src, dst in ((W_v, wv), (W_h, wh), (W_s, wst)):
                stg = cstg.tile([128, 4, H], F32, tag="wstg")
                nc.sync.dma_start(stg[:], src.rearrange("(ki p) ho -> p ki ho",
                                                        p=128))
                nc.any.tensor_copy(dst[:], stg[:])
            for src, dst in ((w_z, wzc), (w_beta, wbc)):
                stg = cstg.tile([128, 4], F32, tag="vstg")
                nc.sync.dma_start(stg[:], src.rearrange("(ht p) -> p ht", p=128))
                nc.any.tensor_copy(dst[:], stg[:])

        bvt = consts.tile([128, 4], F32)
        nc.sync.dma_start(bvt[:], b_v.rearrange("(ht p) -> p ht", p=128))
        bht = consts.tile([128, 4], F32)
        nc.sync.dma_start(bht[:], b_h.rearrange("(ht p) -> p ht", p=128))
        bst = consts.tile([128, 4], F32)
        nc.sync.dma_start(bst[:], b_s.rearrange("(ht p) -> p ht", p=128))
        bvh = consts.tile([128, 4], F32)             # b_v + b_h
        nc.vector.tensor_add(bvh[:], bvt[:], bht[:])
        bsb = consts.tile([128, 4], F32)             # sqrt(.5) * (b_s + b_h)
        nc.vector.tensor_add(bsb[:], bst[:], bht[:])
        nc.scalar.mul(bsb[:], bsb[:], SQ5)

        # softmax logits are shift-invariant: [z+b_z, beta+b_beta] ~ [z, beta+(b_beta-b_z)]
        bzt = consts.tile([1, 1], F32)
        nc.sync.dma_start(bzt[:], b_z.unsqueeze(0))
        bbr = consts.tile([1, 1], F32)               # b_beta - b_z
        nc.sync.dma_start(bbr[:], b_beta.unsqueeze(0))
        nc.vector.tensor_sub(bbr[:], bbr[:], bzt[:])

        identb = consts.tile([128, 128], BF16)
        masks.make_identity(nc, identb[:])
        identf = consts.tile([128, 128], F32)
        masks.make_identity(nc, identf[:])

        # e8[b', (bb,t)] = 1 iff b'==bb: identity rows broadcast t-wise
        # (padded to 32 partitions; rows 8-31 stay zero)
        e8 = consts.tile([32, NCOL], BF16)           # selector mask
        nc.vector.memset(e8[:], 0.0)
        nc.vector.tensor_copy(
            e8[0:8, :].rearrange("p (bb t) -> p bb t", t=TR),
            identb[0:8, 0:8].unsqueeze(2).to_broadcast((8, 8, TR)))

        hpT = consts.tile([128, 4, B], F32)          # (h @ W_h)^T   [ho, b]
        hp8 = consts.tile([32, NCH, H], BF16)        # hp rows staged b%8 (pad 32)
        nc.vector.memset(hp8[:], 0.0)

        beta_row = consts.tile([1, B], BF16)         # beta logits

        # --- preamble: hT, sT, hp (both layouts), beta ----------------------
        with ExitStack() as pre:
            prep = pre.enter_context(tc.tile_pool(name="prep", bufs=2))
            pps = pre.enter_context(tc.tile_pool(name="pps", bufs=2, space="PSUM"))

            hT = prep.tile([128, 4, B], BF16, tag="hT")
            sT = prep.tile([128, 4, B], BF16, tag="hT")
            for src, dst in ((hh, hT), (ss, sT)):
                nat = prep.tile([128, NBT, H], F32, tag="nat")
                nc.sync.dma_start(
                    nat[0:P0, :, :], src.rearrange("(bt p) hx -> p bt hx", p=P0))
                for bt in range(NBT):
                    for ht in range(4):
                        pst = pps.tile([128, 512], F32, tag="tp")
                        nc.tensor.transpose(
                            pst[:, 0:P0], nat[0:P0, bt, ht * 128:(ht + 1) * 128],
                            identf[0:P0, 0:P0])
                        nc.vector.tensor_copy(
                            dst[:, ht, bt * P0:(bt + 1) * P0], pst[:, 0:P0])

            for ht in range(4):
                ps = pps.tile([128, 512], F32, tag="mm")
                for ki in range(4):
                    nc.tensor.matmul(ps[:, 0:B], wh[:, ki, ht * 128:(ht + 1) * 128],
                                     hT[:, ki, :], start=(ki == 0), stop=(ki == 3))
                nc.vector.tensor_copy(hpT[:, ht, :], ps[:, 0:B])

            # hp in natural orientation [b, ho], staged to DRAM and reloaded
            # with partition = b%8 so it can be a K=8 matmul stationary.
            hpn = prep.tile([128, NBT, H], BF16, tag="hpn")
            for bt in range(NBT):
                psn = pps.tile([128, 512], F32, tag="mm")
                for ki in range(4):
                    nc.tensor.matmul(psn[0:P0, 0:H],
                                     hT[:, ki, bt * P0:(bt + 1) * P0],
                                     wh[:, ki, :], start=(ki == 0), stop=(ki == 3))
                nc.vector.tensor_copy(hpn[0:P0, bt, :], psn[0:P0, 0:H])
            nc.sync.dma_start(hpd.rearrange("(bt p) ho -> p bt ho", p=P0),
                              hpn[0:P0, :, :])
            nc.sync.dma_start(hp8[0:8, :, :],
                              hpd.rearrange("(ch p) ho -> p ch ho", p=8))

            betaT = prep.tile([128, 4, B], BF16, tag="betaT")
            for ht in range(4):
                ps = pps.tile([128, 512], F32, tag="mm")
                for ki in range(4):
                    nc.tensor.matmul(ps[:, 0:B], wst[:, ki, ht * 128:(ht + 1) * 128],
                                     sT[:, ki, :], start=(ki == 0), stop=(ki == 3))
                tmp = prep.tile([128, B], F32, tag="btmp")
                nc.vector.tensor_add(tmp[:], ps[:, 0:B], hpT[:, ht, :])
                nc.scalar.activation(betaT[:, ht, :], tmp[:], AF.Tanh,
                                     bias=bsb[:, ht:ht + 1], scale=SQ5)
            psb = pps.tile([128, 512], F32, tag="mmb")
            for ht in range(4):
                nc.tensor.matmul(psb[0:1, 0:B], wbc[:, ht:ht + 1], betaT[:, ht, :],
                                 start=(ht == 0), stop=(ht == 3))
            nc.scalar.activation(beta_row[:], psb[0:1, 0:B], AF.Identity,
                                 bias=bbr[0:1, 0:1])
            # park beta logits in DRAM scratch column 49
            nc.sync.dma_start(zl[:, T:T + 1].rearrange("b o -> o b"),
                              beta_row[:])

        # --- main loop ------------------------------------------------------
        v2_pool = ctx.enter_context(tc.tile_pool(name="v2", bufs=1))
        vt_pool = ctx.enter_context(tc.tile_pool(name="vt", bufs=5))
        z3b_pool = ctx.enter_context(tc.tile_pool(name="z3b", bufs=3))
        sm_pool = ctx.enter_context(tc.tile_pool(name="sm", bufs=2))
        cst_pool = ctx.enter_context(tc.tile_pool(name="cst", bufs=1))
        mm_ps = ctx.enter_context(tc.tile_pool(name="mmps", bufs=4, space="PSUM"))
        z_ps = ctx.enter_context(tc.tile_pool(name="zps", bufs=2, space="PSUM"))
        c_ps = ctx.enter_context(tc.tile_pool(name="cps", bufs=2, space="PSUM"))

        # pinned bf16 v2 tiles (manual rotation): memset once so the dead pad
        # rows (t 50..63 per parity) stay zero -- op2's masked aT relies on
        # multiplying them by 0.0 without NaN surprises.
        v2_tiles = [v2_pool.tile([128, 4, H], BF16, name=f"v2_{i}",
                                 tag=f"v2_{i}") for i in range(N_V2)]
        for t_ in v2_tiles:
            nc.vector.memset(t_[:], 0.0)
        if not USE_SWDGE:
            v2f_tiles = [v2_pool.tile([128, 4, H], F32, name=f"v2f_{i}",
                                      tag=f"v2f_{i}") for i in range(4)]
            for t_ in v2f_tiles:
                nc.vector.memset(t_[:], 0.0)

        # pinned aT tiles: zeros outside the valid parity row ranges mask the
        # opposite parity in the full-K op2 matmuls
        aT0 = sm_pool.tile([128, 32], BF16)
        aT1 = sm_pool.tile([128, 32], BF16)
        aT_tiles = [aT0, aT1]
        for t_ in aT_tiles:
            nc.vector.memset(t_[:], 0.0)
        # c staging: bf16, 4 chunk slots (one group) per store round; free
        # dim padded to 640 so DMA AP lowering can't flat-merge adjacent
        # partition rows into a (wrong) within-partition free run
        cbf = cst_pool.tile([128, 4, 640], BF16)
        nc.vector.memset(cbf[:], 0.0)
        cg = c.rearrange("(ch bb) hx -> bb ch hx", bb=NB)

        for grp in range(NGRP):
            zps = z_ps.tile([128, 512], F32)
            v2cs = []
            for cj in range(4):
                ci = grp * 4 + cj
                b0 = ci * NB

                # 1. SWDGE cast-load v chunk fp32->bf16 into v2 layout
                #    (s injected at t-row 49); one DMA for v, one for s.
                v2c = v2_tiles[ci % N_V2]
                v2cs.append(v2c)
                v2cv = v2c[:].rearrange("(par tp) slot hx -> par tp slot hx",
                                        tp=TP)
                vsrc = v[b0:b0 + NB].rearrange(
                    "(slot par) t hx -> par t slot hx", par=2)
                ssrc = ss[b0:b0 + NB].rearrange(
                    "(slot par) hx -> par slot hx", par=2)
                if USE_SWDGE:
                    for par in (0, 1):
                        nc.gpsimd.dma_start(v2cv[par, 0:T], vsrc[par])
                        nc.gpsimd.dma_start(v2cv[par, T:T + 1],
                                            ssrc[par].unsqueeze(0))
                else:
                    v2f = v2f_tiles[ci % 4]
                    v2fv = v2f[:].rearrange(
                        "(par tp) slot hx -> par tp slot hx", tp=TP)
                    for par in (0, 1):
                        nc.sync.dma_start(v2fv[par, 0:T], vsrc[par])
                        nc.sync.dma_start(v2fv[par, T:T + 1],
                                          ssrc[par].unsqueeze(0))
                    if ci % 2 == 0:
                        nc.vector.tensor_copy(v2c[:], v2f[:])
                    else:
                        nc.scalar.copy(v2c[:], v2f[:])

                # 2. xbar transpose -> vT chunk [128, (slot,ht), 64par+t]
                vtc = vt_pool.tile([128, 16, 128], BF16)
                nc.sync.dma_start_transpose(
                    vtc[:], v2c[:].rearrange("p a b -> p (a b)"))

                # 3+4. main matmul (+hp via selector matmul), tanh from PSUM
                z3b = z3b_pool.tile([128, 4, NCOL], BF16)
                rhs = vtc[:].rearrange(
                    "p (slot ht) tq -> p ht slot tq", ht=4).rearrange(
                    "p ht slot (par t) -> p ht slot par t", par=2)
                hpb = hpT[:, :, b0:b0 + NB].rearrange(
                    "p ho (slot par) -> p ho slot par", par=2).unsqueeze(4)
                for ho in range(4):
                    ps = mm_ps.tile([128, 512], F32, tag="mm")
                    psv = ps[:, 0:NCOL].rearrange(
                        "p (slot par t) -> p slot par t", slot=4, par=2)
                    for ki in range(4):
                        nc.tensor.matmul(
                            psv,
                            wv[:, ki, ho * 128:(ho + 1) * 128],
                            rhs[:, ki, :, :, 0:TR],
                            start=(ki == 0), stop=(False if USE_SEL else ki == 3))
                    if USE_SEL:
                        nc.tensor.matmul(
                            ps[:, 0:NCOL],
                            hp8[:, ci, ho * 128:(ho + 1) * 128],
                            e8[:],
                            start=False, stop=True)
                        nc.scalar.activation(z3b[:, ho, :], ps[:, 0:NCOL],
                                             AF.Tanh, bias=bvh[:, ho:ho + 1])
                    else:
                        z3p = z3b_pool.tile([128, NCOL], F32, tag="z3p")
                        nc.vector.tensor_tensor(
                            z3p[:].rearrange("p (slot par t) -> p slot par t",
                                             slot=4, par=2),
                            psv,
                            hpb[:, ho].to_broadcast((128, 4, 2, TR)),
                            ALU.add)
                        nc.scalar.activation(z3b[:, ho, :], z3p[:], AF.Tanh,
                                             bias=bvh[:, ho:ho + 1])

                # 5. z-reduction into psum row 32*cj
                for ht in range(4):
                    nc.tensor.matmul(zps[32 * cj:32 * cj + 1, 0:NCOL],
                                     wzc[:, ht:ht + 1], z3b[:, ht, :],
                                     start=(ht == 0), stop=(ht == 3),
                                     tile_position=(0, 32 * cj))

            # 6. drain z rows (one full-tile copy), park in DRAM, reload [32,50]
            zst = sm_pool.tile([128, NCOL], BF16, tag="zst")
            nc.vector.tensor_copy(zst[:], zps[:, 0:NCOL])
            nc.sync.dma_start(
                zl[grp * 32:(grp + 1) * 32, 0:T].rearrange(
                    "(cj slot par) t -> cj slot par t", slot=4, par=2),
                zst[:].rearrange(
                    "(cj r) (slot par t) -> cj r slot par t",
                    r=32, slot=4, par=2)[:, 0, :, :, 0:T])
            zg = sm_pool.tile([128, 64], BF16, tag="zg")
            nc.sync.dma_start(zg[0:32, 0:T + 1], zl[grp * 32:(grp + 1) * 32, :])

            # 7. softmax over 50 logits for 32 rows
            negm = sm_pool.tile([128, 1], F32, tag="negm")
            nc.vector.tensor_reduce(negm[0:32], zg[0:32, 0:T + 1], axis=AX.X,
                                    op=ALU.max, negate=True)
            ea = sm_pool.tile([128, T + 1], F32, tag="ea")
            nc.scalar.activation(ea[0:32, :], zg[0:32, 0:T + 1], AF.Exp,
                                 bias=negm[0:32, 0:1])
            ssum = sm_pool.tile([128, 1], F32, tag="ssum")
            nc.vector.tensor_reduce(ssum[0:32], ea[0:32, :], axis=AX.X,
                                    op=ALU.add)
            rinv = sm_pool.tile([128, 1], F32, tag="rinv")
            nc.vector.reciprocal(rinv[0:32], ssum[0:32])
            ab = sm_pool.tile([128, T + 1], BF16, tag="ab")
            nc.vector.tensor_scalar_mul(ab[0:32, :], ea[0:32, :],
                                        rinv[0:32, 0:1])

            # 8. transpose a -> aT columns (col = row-in-group bg; even bg
            # valid on partitions 0..49, odd bg on 64..113, zeros elsewhere)
            aT = aT_tiles[grp % 2]
            pa0 = mm_ps.tile([128, 1024], BF16, tag="mm")
            nc.tensor.matmul(pa0[0:T + 1, 0:32], ab[0:32, :], identb[0:32, 0:32],
                             is_transpose=True, tile_position=(0, 0))
            nc.vector.tensor_copy(
                aT[0:T + 1, :].rearrange("p (c2 par) -> p c2 par", par=2)[:, :, 0],
                pa0[0:T + 1, 0:32].rearrange("p (c2 par) -> p c2 par", par=2)[:, :, 0])
            pa1 = mm_ps.tile([128, 1024], BF16, tag="mm")
            nc.tensor.matmul(pa1[64:64 + T + 1, 0:32], ab[0:32, :],
                             identb[0:32, 0:32],
                             is_transpose=True, tile_position=(0, 64))
            nc.vector.tensor_copy(
                aT[64:64 + T + 1, :].rearrange(
                    "p (c2 par) -> p c2 par", par=2)[:, :, 1],
                pa1[64:64 + T + 1, 0:32].rearrange(
                    "p (c2 par) -> p c2 par", par=2)[:, :, 1])

            # 9. attention apply (op2): one M=2 matmul per slot (both
            # parities packed via zero-masked aT columns), then a single
            # full-tile bf16 drain per chunk and one store DMA per 2 chunks.
            for cj in range(4):
                ci = grp * 4 + cj
                b0 = ci * NB
                v2c = v2cs[cj]
                cps = c_ps.tile([128, H], F32)
                for slot in range(4):
                    bg = cj * 8 + 2 * slot
                    nc.tensor.matmul(
                        cps[32 * slot:32 * slot + 2, :],
                        aT[:, bg:bg + 2],
                        v2c[:, slot, :],
                        start=True, stop=True,
                        tile_position=(0, 32 * slot))
                nc.vector.tensor_copy(cbf[:, cj, 0:H], cps[:])
            for slot in range(4):
                nc.scalar.dma_start(
                    cg[2 * slot:2 * slot + 2, 4 * grp:4 * grp + 4, :],
                    cbf[32 * slot:32 * slot + 2, :, 0:H])

    nc.compile()
    return nc


_NC_CACHE = {}

# test harness hooks: set TRACE=True (with an NTFF profile hook registered)
# to capture HW timing; the BassKernelResults of the last run lands in LAST.
TRACE = False
LAST = {}


def _get_nc(B):
    if B not in _NC_CACHE:
        _NC_CACHE[B] = build_bass(B)
    return _NC_CACHE[B]


def kernel(**inputs):
    from concourse.bass_utils import run_bass_kernel_spmd

    v = np.ascontiguousarray(np.asarray(inputs["v"], dtype=np.float32))
    h = np.ascontiguousarray(np.asarray(inputs["h"], dtype=np.float32))
    s = np.ascontiguousarray(np.asarray(inputs["s"], dtype=np.float32))
    B_total = v.shape[0]
    B = B_total // N_CORES
    nc = _get_nc(B)

    shared = {}
    for k in ("W_h", "b_h", "W_v", "b_v", "w_z", "W_s", "b_s", "w_beta"):
        shared[k] = np.ascontiguousarray(np.asarray(inputs[k], dtype=np.float32))
    for k in ("b_z", "b_beta"):
        shared[k] = np.asarray(inputs[k], dtype=np.float32).reshape(1)

    in_maps = []
    for k in range(N_CORES):
        sl = slice(k * B, (k + 1) * B)
        in_maps.append(dict(shared, v=v[sl], h=h[sl], s=s[sl]))

    kwargs = {"trace": True} if TRACE else {}
    res = run_bass_kernel_spmd(nc, in_maps, core_ids=list(range(N_CORES)),
                               **kwargs)
    LAST["res"] = res
    out = np.concatenate([r["c"] for r in res.results], axis=0)
    return out.astype(np.float32)


# revision 21
# speedup vs baseline: 1.1662x; 1.1285x over previous
"""Trainium2 Bass kernel for nn_Attention_35854386987485 (v2).

Math (per batch row b):
    hp   = h @ W_h                               (bias folded later)
    z3   = tanh(v[b,t] @ W_v + hp + (b_v+b_h))   [T, H]
    z    = z3 @ w_z + b_z                        [T]
    beta = tanh((s @ W_s + hp + (b_s+b_h)) * sqrt(.5)) @ w_beta + b_beta
    a    = softmax([z, beta])                    [T+1]
    c    = sum_t a_t * [v; s][t]                 [H]

Data-parallel over batch across 8 NeuronCores; each core processes B=512 rows.

v2 changes vs baseline (trace-driven):
  * v is cast-loaded fp32->bf16 during DMA (SWDGE) straight into the v2
    layout -- no fp32 staging tiles, no engine-side casts.
  * The hp broadcast-add moved off DVE onto PE: a rank-8 "selector"
    matmul (stationary = this chunk's 8 hp rows staged at partitions
    0-7, moving = a constant 0/1 expansion mask) accumulates hp into
    the same PSUM as the W_v matmuls.  ACT then does tanh straight
    from PSUM with the (b_v+b_h) per-partition bias.
  * Main-matmul moving AP streams runs of 50 (t=0..49 incl. the s row)
    instead of 49 -- even element count keeps the bf16 xbus at full
    rate.  N per chunk is 400 with 8 junk columns (t=49) never read.
  * PSUM drains are single full-tile copies (z: 1 DVE copy/group,
    c: 1 DVE bf16 copy/chunk) instead of per-row copies.
  * z logits / c output round-trip in bf16; c is cast to fp32 on host.
"""

import os
import sys
from contextlib import ExitStack

sys.path.insert(0, "/opt/trn_rl_repo")

import numpy as np

import concourse.bass as bass
import concourse.bacc as bacc
import concourse.tile as tile
from concourse import masks, mybir

F32 = mybir.dt.float32
BF16 = mybir.dt.bfloat16
AF = mybir.ActivationFunctionType
ALU = mybir.AluOpType
AX = mybir.AxisListType

T = 49
H = 512
NB = 8           # batch rows per chunk
TP = 64          # padded t rows per parity in the v2 layout (row 49 = s)
TR = 50          # streamed t rows per parity (incl. s row -> even runs)
NCOL = NB * TR   # packed (slot,par,t50) columns per chunk = 400
SQ5 = float(np.sqrt(0.5))

N_CORES = 8
B_TOTAL = 4096
N_V2 = 12        # pinned v2 chunk tiles in flight
USE_SWDGE = int(os.environ.get("USE_SWDGE", "1"))   # bisect flag
USE_SEL = int(os.environ.get("USE_SEL", "1"))       # bisect flag


def build_bass(B):
    """Build the per-core Bass program for per-core batch size B (mult of 32)."""
    assert B % 32 == 0
    NCH = B // NB          # chunks
    NGRP = NCH // 4        # softmax groups (32 rows each)
    P0 = min(B, 128)       # h/s natural-tile partition count
    NBT = max(B // 128, 1)  # 128-row tiles of h/s
    assert B <= 128 or B % 128 == 0

    nc = bacc.Bacc("TRN2", target_bir_lowering=False, debug=False,
                   num_devices=N_CORES)

    v = nc.dram_tensor("v", (B, T, H), F32, kind="ExternalInput").ap()
    hh = nc.dram_tensor("h", (B, H), F32, kind="ExternalInput").ap()
    ss = nc.dram_tensor("s", (B, H), F32, kind="ExternalInput").ap()
    W_h = nc.dram_tensor("W_h", (H, H), F32, kind="ExternalInput").ap()
    b_h = nc.dram_tensor("b_h", (H,), F32, kind="ExternalInput").ap()
    W_v = nc.dram_tensor("W_v", (H, H), F32, kind="ExternalInput").ap()
    b_v = nc.dram_tensor("b_v", (H,), F32, kind="ExternalInput").ap()
    w_z = nc.dram_tensor("w_z", (H,), F32, kind="ExternalInput").ap()
    b_z = nc.dram_tensor("b_z", (1,), F32, kind="ExternalInput").ap()
    W_s = nc.dram_tensor("W_s", (H, H), F32, kind="ExternalInput").ap()
    b_s = nc.dram_tensor("b_s", (H,), F32, kind="ExternalInput").ap()
    w_beta = nc.dram_tensor("w_beta", (H,), F32, kind="ExternalInput").ap()
    b_beta = nc.dram_tensor("b_beta", (1,), F32, kind="ExternalInput").ap()
    c = nc.dram_tensor("c", (B, H), BF16, kind="ExternalOutput").ap()
    zl = nc.dram_tensor("zl_scratch", (B, T + 1), BF16,
                        kind="ExternalOutput" if os.environ.get("DBG_ZL") else "Internal").ap()
    hpd = nc.dram_tensor("hp_scratch", (B, H), BF16, kind="Internal").ap()

    with tile.TileContext(nc) as tc, ExitStack() as ctx:
        consts = ctx.enter_context(tc.tile_pool(name="consts", bufs=1))

        # --- constant loads (HWDGE fp32, engine-side bf16 casts) ------------
        wv = consts.tile([128, 4, H], BF16)          # W_v[ki*128+p, ho]
        wh = consts.tile([128, 4, H], BF16)
        wst = consts.tile([128, 4, H], BF16)
        wzc = consts.tile([128, 4], BF16)            # w_z[ht*128+p]
        wbc = consts.tile([128, 4], BF16)            # w_beta[ht*128+p]
        with ExitStack() as cpre:
            cstg = cpre.enter_context(tc.tile_pool(name="cstg", bufs=2))
            for src, dst in ((W_v, wv), (W_h, wh), (W_s, wst)):
                stg = cstg.tile([128, 4, H], F32, tag="wstg")
                nc.sync.dma_start(stg[:], src.rearrange("(ki p) ho -> p ki ho",
                                                        p=128))
                nc.any.tensor_copy(dst[:], stg[:])
            for src, dst in ((w_z, wzc), (w_beta, wbc)):
                stg = cstg.tile([128, 4], F32, tag="vstg")
                nc.sync.dma_start(stg[:], src.rearrange("(ht p) -> p ht", p=128))
                nc.any.tensor_copy(dst[:], stg[:])

        bvt = consts.tile([128, 4], F32)
        nc.sync.dma_start(bvt[:], b_v.rearrange("(ht p) -> p ht", p=128))
        bht = consts.tile([128, 4], F32)
        nc.sync.dma_start(bht[:], b_h.rearrange("(ht p) -> p ht", p=128))
        bst = consts.tile([128, 4], F32)
        nc.sync.dma_start(bst[:], b_s.rearrange("(ht p) -> p ht", p=128))
        bvh = consts.tile([128, 4], F32)             # b_v + b_h
        nc.vector.tensor_add(bvh[:], bvt[:], bht[:])
        bsb = consts.tile([128, 4], F32)             # sqrt(.5) * (b_s + b_h)
        nc.vector.tensor_add(bsb[:], bst[:], bht[:])
        nc.scalar.mul(bsb[:], bsb[:], SQ5)

        # softmax logits are shift-invariant: [z+b_z, beta+b_beta] ~ [z, beta+(b_beta-b_z)]
        bzt = consts.tile([1, 1], F32)
        nc.sync.dma_start(bzt[:], b_z.unsqueeze(0))
        bbr = consts.tile([1, 1], F32)               # b_beta - b_z
        nc.sync.dma_start(bbr[:], b_beta.unsqueeze(0))
        nc.vector.tensor_sub(bbr[:], bbr[:], bzt[:])

        identb = consts.tile([128, 128], BF16)
        masks.make_identity(nc, identb[:])
        identf = consts.tile([128, 128], F32)
        masks.make_identity(nc, identf[:])

        # e8[b', (bb,t)] = 1 iff b'==bb: identity rows broadcast t-wise
        # (padded to 32 partitions; rows 8-31 stay zero)
        e8 = consts.tile([32, NCOL], BF16)           # selector mask
        nc.vector.memset(e8[:], 0.0)
        nc.vector.tensor_copy(
            e8[0:8, :].rearrange("p (bb t) -> p bb t", t=TR),
            identb[0:8, 0:8].unsqueeze(2).to_broadcast((8, 8, TR)))

        hpT = consts.tile([128, 4, B], F32)          # (h @ W_h)^T   [ho, b]
        hp8 = consts.tile([32, NCH, H], BF16)        # hp rows staged b%8 (pad 32)
        nc.vector.memset(hp8[:], 0.0)

        beta_row = consts.tile([1, B], BF16)         # beta logits

        # --- preamble: hT, sT, hp (both layouts), beta ----------------------
        with ExitStack() as pre:
            prep = pre.enter_context(tc.tile_pool(name="prep", bufs=2))
            pps = pre.enter_context(tc.tile_pool(name="pps", bufs=2, space="PSUM"))

            hT = prep.tile([128, 4, B], BF16, tag="hT")
            sT = prep.tile([128, 4, B], BF16, tag="hT")
            for src, dst in ((hh, hT), (ss, sT)):
                nat = prep.tile([128, NBT, H], F32, tag="nat")
                nc.sync.dma_start(
                    nat[0:P0, :, :], src.rearrange("(bt p) hx -> p bt hx", p=P0))
                for bt in range(NBT):
                    for ht in range(4):
                        pst = pps.tile([128, 512], F32, tag="tp")
                        nc.tensor.transpose(
                            pst[:, 0:P0], nat[0:P0, bt, ht * 128:(ht + 1) * 128],
                            identf[0:P0, 0:P0])
                        nc.vector.tensor_copy(
                            dst[:, ht, bt * P0:(bt + 1) * P0], pst[:, 0:P0])

            for ht in range(4):
                ps = pps.tile([128, 512], F32, tag="mm")
                for ki in range(4):
                    nc.tensor.matmul(ps[:, 0:B], wh[:, ki, ht * 128:(ht + 1) * 128],
                                     hT[:, ki, :], start=(ki == 0), stop=(ki == 3))
                nc.vector.tensor_copy(hpT[:, ht, :], ps[:, 0:B])

            # hp in natural orientation [b, ho], staged to DRAM and reloaded
            # with partition = b%8 so it can be a K=8 matmul stationary.
            hpn = prep.tile([128, NBT, H], BF16, tag="hpn")
            for bt in range(NBT):
                psn = pps.tile([128, 512], F32, tag="mm")
                for ki in range(4):
                    nc.tensor.matmul(psn[0:P0, 0:H],
                                     hT[:, ki, bt * P0:(bt + 1) * P0],
                                     wh[:, ki, :], start=(ki == 0), stop=(ki == 3))
                nc.vector.tensor_copy(hpn[0:P0, bt, :], psn[0:P0, 0:H])
            nc.sync.dma_start(hpd.rearrange("(bt p) ho -> p bt ho", p=P0),
                              hpn[0:P0, :, :])
            nc.sync.dma_start(hp8[0:8, :, :],
                              hpd.rearrange("(ch p) ho -> p ch ho", p=8))

            betaT = prep.tile([128, 4, B], BF16, tag="betaT")
            for ht in range(4):
                ps = pps.tile([128, 512], F32, tag="mm")
                for ki in range(4):
                    nc.tensor.matmul(ps[:, 0:B], wst[:, ki, ht * 128:(ht + 1) * 128],
                                     sT[:, ki, :], start=(ki == 0), stop=(ki == 3))
                tmp = prep.tile([128, B], F32, tag="btmp")
                nc.vector.tensor_add(tmp[:], ps[:, 0:B], hpT[:, ht, :])
                nc.scalar.activation(betaT[:, ht, :], tmp[:], AF.Tanh,
                                     bias=bsb[:, ht:ht + 1], scale=SQ5)
            psb = pps.tile([128, 512], F32, tag="mmb")
            for ht in range(4):
                nc.tensor.matmul(psb[0:1, 0:B], wbc[:, ht:ht + 1], betaT[:, ht, :],
                                 start=(ht == 0), stop=(ht == 3))
            nc.scalar.activation(beta_row[:], psb[0:1, 0:B], AF.Identity,
                                 bias=bbr[0:1, 0:1])
            # park beta logits in DRAM scratch column 49
            nc.sync.dma_start(zl[:, T:T + 1].rearrange("b o -> o b"),
                              beta_row[:])

        # --- main loop ------------------------------------------------------
        v2_pool = ctx.enter_context(tc.tile_pool(name="v2", bufs=1))
        vt_pool = ctx.enter_context(tc.tile_pool(name="vt", bufs=5))
        z3b_pool = ctx.enter_context(tc.tile_pool(name="z3b", bufs=3))
        sm_pool = ctx.enter_context(tc.tile_pool(name="sm", bufs=2))
        cst_pool = ctx.enter_context(tc.tile_pool(name="cst", bufs=1))
        mm_ps = ctx.enter_context(tc.tile_pool(name="mmps", bufs=4, space="PSUM"))
        z_ps = ctx.enter_context(tc.tile_pool(name="zps", bufs=2, space="PSUM"))
        c_ps = ctx.enter_context(tc.tile_pool(name="cps", bufs=2, space="PSUM"))

        # pinned bf16 v2 tiles (manual rotation): memset once so the dead pad
        # rows (t 50..63 per parity) stay zero -- op2's masked aT relies on
        # multiplying them by 0.0 without NaN surprises.
        v2_tiles = [v2_pool.tile([128, 4, H], BF16, name=f"v2_{i}",
                                 tag=f"v2_{i}") for i in range(N_V2)]
        for t_ in v2_tiles:
            nc.vector.memset(t_[:], 0.0)
        if not USE_SWDGE:
            v2f_tiles = [v2_pool.tile([128, 4, H], F32, name=f"v2f_{i}",
                                      tag=f"v2f_{i}") for i in range(4)]
            for t_ in v2f_tiles:
                nc.vector.memset(t_[:], 0.0)

        # pinned aT tiles: zeros outside the valid parity row ranges mask the
        # opposite parity in the full-K op2 matmuls
        aT0 = sm_pool.tile([128, 32], BF16)
        aT1 = sm_pool.tile([128, 32], BF16)
        aT_tiles = [aT0, aT1]
        for t_ in aT_tiles:
            nc.vector.memset(t_[:], 0.0)
        # c staging: bf16, 4 chunk slots (one group) per store round; free
        # dim padded to 640 so DMA AP lowering can't flat-merge adjacent
        # partition rows into a (wrong) within-partition free run
        cbf = cst_pool.tile([128, 4, 640], BF16)
        nc.vector.memset(cbf[:], 0.0)
        cg = c.rearrange("(ch bb) hx -> bb ch hx", bb=NB)

        for grp in range(NGRP):
            zps = z_ps.tile([128, 512], F32)
            v2cs = []
            for cj in range(4):
                ci = grp * 4 + cj
                b0 = ci * NB

                # 1. SWDGE cast-load v chunk fp32->bf16 into v2 layout
                #    (s injected at t-row 49); one DMA for v, one for s.
                v2c = v2_tiles[ci % N_V2]
                v2cs.append(v2c)
                v2cv = v2c[:].rearrange("(par tp) slot hx -> par tp slot hx",
                                        tp=TP)
                vsrc = v[b0:b0 + NB].rearrange(
                    "(slot par) t hx -> par t slot hx", par=2)
                ssrc = ss[b0:b0 + NB].rearrange(
                    "(slot par) hx -> par slot hx", par=2)
                if USE_SWDGE:
                    for par in (0, 1):
                        nc.gpsimd.dma_start(v2cv[par, 0:T], vsrc[par])
                        nc.gpsimd.dma_start(v2cv[par, T:T + 1],
                                            ssrc[par].unsqueeze(0))
                else:
                    v2f = v2f_tiles[ci % 4]
                    v2fv = v2f[:].rearrange(
                        "(par tp) slot hx -> par tp slot hx", tp=TP)
                    for par in (0, 1):
                        nc.sync.dma_start(v2fv[par, 0:T], vsrc[par])
                        nc.sync.dma_start(v2fv[par, T:T + 1],
                                          ssrc[par].unsqueeze(0))
                    if ci % 2 == 0:
                        nc.vector.tensor_copy(v2c[:], v2f[:])
                    else:
                        nc.scalar.copy(v2c[:], v2f[:])

                # 2. xbar transpose -> vT chunk [128, (slot,ht), 64par+t]
                vtc = vt_pool.tile([128, 16, 128], BF16)
                nc.sync.dma_start_transpose(
                    vtc[:], v2c[:].rearrange("p a b -> p (a b)"))

                # 3+4. main matmul (+hp via selector matmul), tanh from PSUM
                z3b = z3b_pool.tile([128, 4, NCOL], BF16)
                rhs = vtc[:].rearrange(
                    "p (slot ht) tq -> p ht slot tq", ht=4).rearrange(
                    "p ht slot (par t) -> p ht slot par t", par=2)
                hpb = hpT[:, :, b0:b0 + NB].rearrange(
                    "p ho (slot par) -> p ho slot par", par=2).unsqueeze(4)
                for ho in range(4):
                    ps = mm_ps.tile([128, 512], F32, tag="mm")
                    psv = ps[:, 0:NCOL].rearrange(
                        "p (slot par t) -> p slot par t", slot=4, par=2)
                    for ki in range(4):
                        nc.tensor.matmul(
                            psv,
                            wv[:, ki, ho * 128:(ho + 1) * 128],
                            rhs[:, ki, :, :, 0:TR],
                            start=(ki == 0), stop=(False if USE_SEL else ki == 3))
                    if USE_SEL:
                        nc.tensor.matmul(
                            ps[:, 0:NCOL],
                            hp8[:, ci, ho * 128:(ho + 1) * 128],
                            e8[:],
                            start=False, stop=True)
                        nc.scalar.activation(z3b[:, ho, :], ps[:, 0:NCOL],
                                             AF.Tanh, bias=bvh[:, ho:ho + 1])
                    else:
                        z3p = z3b_pool.tile([128, NCOL], F32, tag="z3p")
                        nc.vector.tensor_tensor(
                            z3p[:].rearrange("p (slot par t) -> p slot par t",
                                             slot=4, par=2),
                            psv,
                            hpb[:, ho].to_broadcast((128, 4, 2, TR)),
                            ALU.add)
                        nc.scalar.activation(z3b[:, ho, :], z3p[:], AF.Tanh,
                                             bias=bvh[:, ho:ho + 1])

                # 5. z-reduction into psum row 32*cj
                for ht in range(4):
                    nc.tensor.matmul(zps[32 * cj:32 * cj + 1, 0:NCOL],
                                     wzc[:, ht:ht + 1], z3b[:, ht, :],
                                     start=(ht == 0), stop=(ht == 3),
                                     tile_position=(0, 32 * cj))

            # 6. drain z rows (one full-tile copy), park in DRAM, reload [32,50]
            zst = sm_pool.tile([128, NCOL], BF16, tag="zst")
            nc.vector.tensor_copy(zst[:], zps[:, 0:NCOL])
            nc.scalar.dma_start(
                zl[grp * 32:(grp + 1) * 32, 0:T].rearrange(
                    "(cj slot par) t -> cj slot par t", slot=4, par=2),
                zst[:].rearrange(
                    "(cj r) (slot par t) -> cj r slot par t",
                    r=32, slot=4, par=2)[:, 0, :, :, 0:T])
            zg = sm_pool.tile([128, 64], BF16, tag="zg")
            nc.scalar.dma_start(zg[0:32, 0:T + 1],
                                zl[grp * 32:(grp + 1) * 32, :])

            # 7. softmax over 50 logits for 32 rows
            negm = sm_pool.tile([128, 1], F32, tag="negm")
            nc.vector.tensor_reduce(negm[0:32], zg[0:32, 0:T + 1], axis=AX.X,
                                    op=ALU.max, negate=True)
            ea = sm_pool.tile([128, T + 1], F32, tag="ea")
            nc.scalar.activation(ea[0:32, :], zg[0:32, 0:T + 1], AF.Exp,
                                 bias=negm[0:32, 0:1])
            ssum = sm_pool.tile([128, 1], F32, tag="ssum")
            nc.vector.tensor_reduce(ssum[0:32], ea[0:32, :], axis=AX.X,
                                    op=ALU.add)
            rinv = sm_pool.tile([128, 1], F32, tag="rinv")
            nc.vector.reciprocal(rinv[0:32], ssum[0:32])
            ab = sm_pool.tile([128, T + 1], BF16, tag="ab")
            nc.vector.tensor_scalar_mul(ab[0:32, :], ea[0:32, :],
                                        rinv[0:32, 0:1])

            # 8. transpose a -> aT columns (col = row-in-group bg; even bg
            # valid on partitions 0..49, odd bg on 64..113, zeros elsewhere)
            aT = aT_tiles[grp % 2]
            pa0 = mm_ps.tile([128, 1024], BF16, tag="mm")
            nc.tensor.matmul(pa0[0:T + 1, 0:32], ab[0:32, :], identb[0:32, 0:32],
                             is_transpose=True, tile_position=(0, 0))
            nc.vector.tensor_copy(
                aT[0:T + 1, :].rearrange("p (c2 par) -> p c2 par", par=2)[:, :, 0],
                pa0[0:T + 1, 0:32].rearrange("p (c2 par) -> p c2 par", par=2)[:, :, 0])
            pa1 = mm_ps.tile([128, 1024], BF16, tag="mm")
            nc.tensor.matmul(pa1[64:64 + T + 1, 0:32], ab[0:32, :],
                             identb[0:32, 0:32],
                             is_transpose=True, tile_position=(0, 64))
            nc.vector.tensor_copy(
                aT[64:64 + T + 1, :].rearrange(
                    "p (c2 par) -> p c2 par", par=2)[:, :, 1],
                pa1[64:64 + T + 1, 0:32].rearrange(
                    "p (c2 par) -> p c2 par", par=2)[:, :, 1])

            # 9. attention apply (op2): one M=2 matmul per slot (both
            # parities packed via zero-masked aT columns), then a single
            # full-tile bf16 drain per chunk and one store DMA per 2 chunks.
            for cj in range(4):
                ci = grp * 4 + cj
                b0 = ci * NB
                v2c = v2cs[cj]
                cps = c_ps.tile([128, H], F32)
                for slot in range(4):
                    bg = cj * 8 + 2 * slot
                    nc.tensor.matmul(
                        cps[32 * slot:32 * slot + 2, :],
                        aT[:, bg:bg + 2],
                        v2c[:, slot, :],
                        start=True, stop=True,
                        tile_position=(0, 32 * slot))
                nc.vector.tensor_copy(cbf[:, cj, 0:H], cps[:])
            for slot in range(4):
                nc.scalar.dma_start(
                    cg[2 * slot:2 * slot + 2, 4 * grp:4 * grp + 4, :],
                    cbf[32 * slot:32 * slot + 2, :, 0:H])

    nc.compile()
    return nc


_NC_CACHE = {}

# test harness hooks: set TRACE=True (with an NTFF profile hook registered)
# to capture HW timing; the BassKernelResults of the last run lands in LAST.
TRACE = False
LAST = {}


def _get_nc(B):
    if B not in _NC_CACHE:
        _NC_CACHE[B] = build_bass(B)
    return _NC_CACHE[B]


def kernel(**inputs):
    from concourse.bass_utils import run_bass_kernel_spmd

    v = np.ascontiguousarray(np.asarray(inputs["v"], dtype=np.float32))
    h = np.ascontiguousarray(np.asarray(inputs["h"], dtype=np.float32))
    s = np.ascontiguousarray(np.asarray(inputs["s"], dtype=np.float32))
    B_total = v.shape[0]
    B = B_total // N_CORES
    nc = _get_nc(B)

    shared = {}
    for k in ("W_h", "b_h", "W_v", "b_v", "w_z", "W_s", "b_s", "w_beta"):
        shared[k] = np.ascontiguousarray(np.asarray(inputs[k], dtype=np.float32))
    for k in ("b_z", "b_beta"):
        shared[k] = np.asarray(inputs[k], dtype=np.float32).reshape(1)

    in_maps = []
    for k in range(N_CORES):
        sl = slice(k * B, (k + 1) * B)
        in_maps.append(dict(shared, v=v[sl], h=h[sl], s=s[sl]))

    kwargs = {"trace": True} if TRACE else {}
    res = run_bass_kernel_spmd(nc, in_maps, core_ids=list(range(N_CORES)),
                               **kwargs)
    LAST["res"] = res
    out = np.concatenate([r["c"] for r in res.results], axis=0)
    return out.astype(np.float32)


# revision 24
# speedup vs baseline: 1.3997x; 1.2002x over previous
"""Trainium2 Bass kernel for nn_Attention_35854386987485 (v2).

Math (per batch row b):
    hp   = h @ W_h                               (bias folded later)
    z3   = tanh(v[b,t] @ W_v + hp + (b_v+b_h))   [T, H]
    z    = z3 @ w_z + b_z                        [T]
    beta = tanh((s @ W_s + hp + (b_s+b_h)) * sqrt(.5)) @ w_beta + b_beta
    a    = softmax([z, beta])                    [T+1]
    c    = sum_t a_t * [v; s][t]                 [H]

Data-parallel over batch across 8 NeuronCores; each core processes B=512 rows.

v2 changes vs baseline (trace-driven):
  * v is cast-loaded fp32->bf16 during DMA (SWDGE) straight into the v2
    layout -- no fp32 staging tiles, no engine-side casts.
  * The hp broadcast-add moved off DVE onto PE: a rank-8 "selector"
    matmul (stationary = this chunk's 8 hp rows staged at partitions
    0-7, moving = a constant 0/1 expansion mask) accumulates hp into
    the same PSUM as the W_v matmuls.  ACT then does tanh straight
    from PSUM with the (b_v+b_h) per-partition bias.
  * Main-matmul moving AP streams runs of 50 (t=0..49 incl. the s row)
    instead of 49 -- even element count keeps the bf16 xbus at full
    rate.  N per chunk is 400 with 8 junk columns (t=49) never read.
  * PSUM drains are single full-tile copies (z: 1 DVE copy/group,
    c: 1 DVE bf16 copy/chunk) instead of per-row copies.
  * z logits / c output round-trip in bf16; c is cast to fp32 on host.
"""

import os
import sys
from contextlib import ExitStack

sys.path.insert(0, "/opt/trn_rl_repo")

import numpy as np

import concourse.bass as bass
import concourse.bacc as bacc
import concourse.tile as tile
from concourse import masks, mybir

F32 = mybir.dt.float32
BF16 = mybir.dt.bfloat16
AF = mybir.ActivationFunctionType
ALU = mybir.AluOpType
AX = mybir.AxisListType

T = 49
H = 512
NB = 8           # batch rows per chunk
TP = 64          # padded t rows per parity in the v2 layout (row 49 = s)
TR = 50          # streamed t rows per parity (incl. s row -> even runs)
NCOL = NB * TR   # packed (slot,par,t50) columns per chunk = 400
SQ5 = float(np.sqrt(0.5))

N_CORES = 8
B_TOTAL = 4096
N_V2 = 16        # pinned v2 chunk tiles in flight
USE_SWDGE = int(os.environ.get("USE_SWDGE", "1"))   # bisect flag
USE_SEL = int(os.environ.get("USE_SEL", "1"))       # bisect flag


def build_bass(B):
    """Build the per-core Bass program for per-core batch size B (mult of 32)."""
    assert B % 32 == 0
    NCH = B // NB          # chunks
    NGRP = NCH // 4        # softmax groups (32 rows each)
    P0 = min(B, 128)       # h/s natural-tile partition count
    NBT = max(B // 128, 1)  # 128-row tiles of h/s
    assert B <= 128 or B % 128 == 0

    nc = bacc.Bacc("TRN2", target_bir_lowering=False, debug=False,
                   num_devices=N_CORES)

    v = nc.dram_tensor("v", (B, T, H), F32, kind="ExternalInput").ap()
    hh = nc.dram_tensor("h", (B, H), F32, kind="ExternalInput").ap()
    ss = nc.dram_tensor("s", (B, H), F32, kind="ExternalInput").ap()
    W_h = nc.dram_tensor("W_h", (H, H), F32, kind="ExternalInput").ap()
    b_h = nc.dram_tensor("b_h", (H,), F32, kind="ExternalInput").ap()
    W_v = nc.dram_tensor("W_v", (H, H), F32, kind="ExternalInput").ap()
    b_v = nc.dram_tensor("b_v", (H,), F32, kind="ExternalInput").ap()
    w_z = nc.dram_tensor("w_z", (H,), F32, kind="ExternalInput").ap()
    b_z = nc.dram_tensor("b_z", (1,), F32, kind="ExternalInput").ap()
    W_s = nc.dram_tensor("W_s", (H, H), F32, kind="ExternalInput").ap()
    b_s = nc.dram_tensor("b_s", (H,), F32, kind="ExternalInput").ap()
    w_beta = nc.dram_tensor("w_beta", (H,), F32, kind="ExternalInput").ap()
    b_beta = nc.dram_tensor("b_beta", (1,), F32, kind="ExternalInput").ap()
    c = nc.dram_tensor("c", (B, H), BF16, kind="ExternalOutput").ap()
    zl = nc.dram_tensor("zl_scratch", (B, T + 1), BF16,
                        kind="ExternalOutput" if os.environ.get("DBG_ZL") else "Internal").ap()
    hpd = nc.dram_tensor("hp_scratch", (B, H), BF16, kind="Internal").ap()

    with tile.TileContext(nc) as tc, ExitStack() as ctx:
        consts = ctx.enter_context(tc.tile_pool(name="consts", bufs=1))

        # --- constant loads (HWDGE fp32, engine-side bf16 casts) ------------
        wv = consts.tile([128, 4, H], BF16)          # W_v[ki*128+p, ho]
        wh = consts.tile([128, 4, H], BF16)
        wst = consts.tile([128, 4, H], BF16)
        wzc = consts.tile([128, 4], BF16)            # w_z[ht*128+p]
        wbc = consts.tile([128, 4], BF16)            # w_beta[ht*128+p]
        with ExitStack() as cpre:
            cstg = cpre.enter_context(tc.tile_pool(name="cstg", bufs=2))
            for src, dst in ((W_v, wv), (W_h, wh), (W_s, wst)):
                stg = cstg.tile([128, 4, H], F32, tag="wstg")
                nc.sync.dma_start(stg[:], src.rearrange("(ki p) ho -> p ki ho",
                                                        p=128))
                nc.any.tensor_copy(dst[:], stg[:])
            for src, dst in ((w_z, wzc), (w_beta, wbc)):
                stg = cstg.tile([128, 4], F32, tag="vstg")
                nc.sync.dma_start(stg[:], src.rearrange("(ht p) -> p ht", p=128))
                nc.any.tensor_copy(dst[:], stg[:])

        bvt = consts.tile([128, 4], F32)
        nc.sync.dma_start(bvt[:], b_v.rearrange("(ht p) -> p ht", p=128))
        bht = consts.tile([128, 4], F32)
        nc.sync.dma_start(bht[:], b_h.rearrange("(ht p) -> p ht", p=128))
        bst = consts.tile([128, 4], F32)
        nc.sync.dma_start(bst[:], b_s.rearrange("(ht p) -> p ht", p=128))
        bvh = consts.tile([128, 4], F32)             # b_v + b_h
        nc.vector.tensor_add(bvh[:], bvt[:], bht[:])
        bsb = consts.tile([128, 4], F32)             # sqrt(.5) * (b_s + b_h)
        nc.vector.tensor_add(bsb[:], bst[:], bht[:])
        nc.scalar.mul(bsb[:], bsb[:], SQ5)

        # softmax logits are shift-invariant: [z+b_z, beta+b_beta] ~ [z, beta+(b_beta-b_z)]
        bzt = consts.tile([1, 1], F32)
        nc.sync.dma_start(bzt[:], b_z.unsqueeze(0))
        bbr = consts.tile([1, 1], F32)               # b_beta - b_z
        nc.sync.dma_start(bbr[:], b_beta.unsqueeze(0))
        nc.vector.tensor_sub(bbr[:], bbr[:], bzt[:])

        identb = consts.tile([128, 128], BF16)
        masks.make_identity(nc, identb[:])
        identf = consts.tile([128, 128], F32)
        masks.make_identity(nc, identf[:])

        # e8[b', (bb,t)] = 1 iff b'==bb: identity rows broadcast t-wise.
        # K=64 stationary: rows 0-7 select hp rows, row 32 is all-ones and
        # pairs with the bvh row of hp8 -- the selector matmul adds
        # hp[b] + (b_v+b_h) into PSUM in one shot.  Rows 8-31/33-63 zero.
        e8 = consts.tile([64, NCOL], BF16)           # selector mask
        nc.vector.memset(e8[:], 0.0)
        nc.vector.tensor_copy(
            e8[0:8, :].rearrange("p (bb t) -> p bb t", t=TR),
            identb[0:8, 0:8].unsqueeze(2).to_broadcast((8, 8, TR)))
        nc.vector.memset(e8[32:33, :], 1.0)

        hpT = consts.tile([128, 4, B], F32)          # (h @ W_h)^T   [ho, b]
        hp8 = consts.tile([64, NCH, H], BF16)        # hp rows b%8 + bvh row 32
        nc.vector.memset(hp8[:], 0.0)
        bvhf = consts.tile([1, H], F32)              # b_v + b_h as one row
        bhf = consts.tile([1, H], F32)
        nc.sync.dma_start(bvhf[:], b_v.unsqueeze(0))
        nc.sync.dma_start(bhf[:], b_h.unsqueeze(0))
        nc.vector.tensor_add(bvhf[:], bvhf[:], bhf[:])
        nc.vector.tensor_copy(
            hp8[32:33, :, :], bvhf.unsqueeze(1).to_broadcast((1, NCH, H)))

        beta_row = consts.tile([1, B], BF16)         # beta logits

        # --- preamble: hT, sT, hp (both layouts), beta ----------------------
        with ExitStack() as pre:
            prep = pre.enter_context(tc.tile_pool(name="prep", bufs=2))
            pps = pre.enter_context(tc.tile_pool(name="pps", bufs=2, space="PSUM"))

            hT = prep.tile([128, 4, B], BF16, tag="hT")
            sT = prep.tile([128, 4, B], BF16, tag="hT")
            for src, dst in ((hh, hT), (ss, sT)):
                nat = prep.tile([128, NBT, H], F32, tag="nat")
                nc.sync.dma_start(
                    nat[0:P0, :, :], src.rearrange("(bt p) hx -> p bt hx", p=P0))
                for bt in range(NBT):
                    for ht in range(4):
                        pst = pps.tile([128, 512], F32, tag="tp")
                        nc.tensor.transpose(
                            pst[:, 0:P0], nat[0:P0, bt, ht * 128:(ht + 1) * 128],
                            identf[0:P0, 0:P0])
                        nc.vector.tensor_copy(
                            dst[:, ht, bt * P0:(bt + 1) * P0], pst[:, 0:P0])

            for ht in range(4):
                ps = pps.tile([128, 512], F32, tag="mm")
                for ki in range(4):
                    nc.tensor.matmul(ps[:, 0:B], wh[:, ki, ht * 128:(ht + 1) * 128],
                                     hT[:, ki, :], start=(ki == 0), stop=(ki == 3))
                nc.vector.tensor_copy(hpT[:, ht, :], ps[:, 0:B])

            # hp in natural orientation [b, ho], staged to DRAM and reloaded
            # with partition = b%8 so it can be a K=8 matmul stationary.
            hpn = prep.tile([128, NBT, H], BF16, tag="hpn")
            for bt in range(NBT):
                psn = pps.tile([128, 512], F32, tag="mm")
                for ki in range(4):
                    nc.tensor.matmul(psn[0:P0, 0:H],
                                     hT[:, ki, bt * P0:(bt + 1) * P0],
                                     wh[:, ki, :], start=(ki == 0), stop=(ki == 3))
                nc.vector.tensor_copy(hpn[0:P0, bt, :], psn[0:P0, 0:H])
            nc.sync.dma_start(hpd.rearrange("(bt p) ho -> p bt ho", p=P0),
                              hpn[0:P0, :, :])
            nc.sync.dma_start(hp8[0:8, :, :],
                              hpd.rearrange("(ch p) ho -> p ch ho", p=8))

            betaT = prep.tile([128, 4, B], BF16, tag="betaT")
            for ht in range(4):
                ps = pps.tile([128, 512], F32, tag="mm")
                for ki in range(4):
                    nc.tensor.matmul(ps[:, 0:B], wst[:, ki, ht * 128:(ht + 1) * 128],
                                     sT[:, ki, :], start=(ki == 0), stop=(ki == 3))
                tmp = prep.tile([128, B], F32, tag="btmp")
                nc.vector.tensor_add(tmp[:], ps[:, 0:B], hpT[:, ht, :])
                nc.scalar.activation(betaT[:, ht, :], tmp[:], AF.Tanh,
                                     bias=bsb[:, ht:ht + 1], scale=SQ5)
            psb = pps.tile([128, 512], F32, tag="mmb")
            for ht in range(4):
                nc.tensor.matmul(psb[0:1, 0:B], wbc[:, ht:ht + 1], betaT[:, ht, :],
                                 start=(ht == 0), stop=(ht == 3))
            nc.scalar.activation(beta_row[:], psb[0:1, 0:B], AF.Identity,
                                 bias=bbr[0:1, 0:1])
            # park beta logits in DRAM scratch column 49
            nc.sync.dma_start(zl[:, T:T + 1].rearrange("b o -> o b"),
                              beta_row[:])

        # --- main loop ------------------------------------------------------
        v2_pool = ctx.enter_context(tc.tile_pool(name="v2", bufs=1))
        vt_pool = ctx.enter_context(tc.tile_pool(name="vt", bufs=6))
        z3b_pool = ctx.enter_context(tc.tile_pool(name="z3b", bufs=3))
        sm_pool = ctx.enter_context(tc.tile_pool(name="sm", bufs=2))
        cst_pool = ctx.enter_context(tc.tile_pool(name="cst", bufs=1))
        mm_ps = ctx.enter_context(tc.tile_pool(name="mmps", bufs=2, space="PSUM"))
        z_ps = ctx.enter_context(tc.tile_pool(name="zps", bufs=2, space="PSUM"))
        c_ps = ctx.enter_context(tc.tile_pool(name="cps", bufs=2, space="PSUM"))

        # pinned bf16 v2 tiles (manual rotation): memset once so the dead pad
        # rows (t 50..63 per parity) stay zero -- op2's masked aT relies on
        # multiplying them by 0.0 without NaN surprises.
        v2_tiles = [v2_pool.tile([128, 4, H], BF16, name=f"v2_{i}",
                                 tag=f"v2_{i}") for i in range(N_V2)]
        for t_ in v2_tiles:
            nc.vector.memset(t_[:], 0.0)
        if not USE_SWDGE:
            v2f_tiles = [v2_pool.tile([128, 4, H], F32, name=f"v2f_{i}",
                                      tag=f"v2f_{i}") for i in range(4)]
            for t_ in v2f_tiles:
                nc.vector.memset(t_[:], 0.0)

        # pinned aT tiles: zeros outside the valid parity row ranges mask the
        # opposite parity in the full-K op2 matmuls
        aT0 = sm_pool.tile([128, 32], BF16)
        aT1 = sm_pool.tile([128, 32], BF16)
        aT_tiles = [aT0, aT1]
        for t_ in aT_tiles:
            nc.vector.memset(t_[:], 0.0)
        # c staging: bf16, 4 chunk slots (one group) per store round; free
        # dim padded to 640 so DMA AP lowering can't flat-merge adjacent
        # partition rows into a (wrong) within-partition free run
        cbf = cst_pool.tile([128, 4, 640], BF16)
        nc.vector.memset(cbf[:], 0.0)
        cg = c.rearrange("(ch bb) hx -> bb ch hx", bb=NB)

        for grp in range(NGRP):
            zps = z_ps.tile([128, 512], F32)
            v2cs = []
            for cj in range(4):
                ci = grp * 4 + cj
                b0 = ci * NB

                # 1. SWDGE cast-load v chunk fp32->bf16 into v2 layout
                #    (s injected at t-row 49); one DMA for v, one for s.
                v2c = v2_tiles[ci % N_V2]
                v2cs.append(v2c)
                v2cv = v2c[:].rearrange("(par tp) slot hx -> par tp slot hx",
                                        tp=TP)
                vsrc = v[b0:b0 + NB].rearrange(
                    "(slot par) t hx -> par t slot hx", par=2)
                ssrc = ss[b0:b0 + NB].rearrange(
                    "(slot par) hx -> par slot hx", par=2)
                if USE_SWDGE:
                    for par in (0, 1):
                        nc.gpsimd.dma_start(v2cv[par, 0:T], vsrc[par])
                        nc.gpsimd.dma_start(v2cv[par, T:T + 1],
                                            ssrc[par].unsqueeze(0))
                else:
                    v2f = v2f_tiles[ci % 4]
                    v2fv = v2f[:].rearrange(
                        "(par tp) slot hx -> par tp slot hx", tp=TP)
                    for par in (0, 1):
                        nc.sync.dma_start(v2fv[par, 0:T], vsrc[par])
                        nc.sync.dma_start(v2fv[par, T:T + 1],
                                          ssrc[par].unsqueeze(0))
                    if ci % 2 == 0:
                        nc.vector.tensor_copy(v2c[:], v2f[:])
                    else:
                        nc.scalar.copy(v2c[:], v2f[:])

                # 2. xbar transpose -> vT chunk [128, (slot,ht), 64par+t]
                vtc = vt_pool.tile([128, 16, 128], BF16)
                nc.sync.dma_start_transpose(
                    vtc[:], v2c[:].rearrange("p a b -> p (a b)"))

                # 3+4. main matmul (+hp via selector matmul), tanh from PSUM
                z3b = z3b_pool.tile([128, 4, NCOL], BF16)
                rhs = vtc[:].rearrange(
                    "p (slot ht) tq -> p ht slot tq", ht=4).rearrange(
                    "p ht slot (par t) -> p ht slot par t", par=2)
                for hop in range(2):
                    # second dim padded to 512 = one PSUM bank per sub --
                    # a matmul output may not cross a bank boundary
                    ps2 = mm_ps.tile([128, 2, 512], F32, tag="mm")
                    for sub in range(2):
                        ho = 2 * hop + sub
                        psv = ps2[:, sub, 0:NCOL].rearrange(
                            "p (slot par t) -> p slot par t", slot=4, par=2)
                        for ki in range(4):
                            nc.tensor.matmul(
                                psv,
                                wv[:, ki, ho * 128:(ho + 1) * 128],
                                rhs[:, ki, :, :, 0:TR],
                                start=(ki == 0), stop=False)
                        nc.tensor.matmul(
                            ps2[:, sub, 0:NCOL],
                            hp8[:, ci, ho * 128:(ho + 1) * 128],
                            e8[:],
                            start=False, stop=True)
                    nc.scalar.activation(
                        z3b[:, 2 * hop:2 * hop + 2, :],
                        ps2[:, :, 0:NCOL], AF.Tanh)

                # 5. z-reduction into psum row 32*cj
                for ht in range(4):
                    nc.tensor.matmul(zps[32 * cj:32 * cj + 1, 0:NCOL],
                                     wzc[:, ht:ht + 1], z3b[:, ht, :],
                                     start=(ht == 0), stop=(ht == 3),
                                     tile_position=(0, 32 * cj))

            # 6. drain z rows (one full-tile copy), park in DRAM, reload [32,50]
            zst = sm_pool.tile([128, NCOL], BF16, tag="zst")
            nc.vector.tensor_copy(zst[:], zps[:, 0:NCOL])
            nc.scalar.dma_start(
                zl[grp * 32:(grp + 1) * 32, 0:T].rearrange(
                    "(cj slot par) t -> cj slot par t", slot=4, par=2),
                zst[:].rearrange(
                    "(cj r) (slot par t) -> cj r slot par t",
                    r=32, slot=4, par=2)[:, 0, :, :, 0:T])
            zg = sm_pool.tile([128, 64], BF16, tag="zg")
            nc.scalar.dma_start(zg[0:32, 0:T + 1],
                                zl[grp * 32:(grp + 1) * 32, :])

            # 7. softmax over 50 logits for 32 rows
            negm = sm_pool.tile([128, 1], F32, tag="negm")
            nc.vector.tensor_reduce(negm[0:32], zg[0:32, 0:T + 1], axis=AX.X,
                                    op=ALU.max, negate=True)
            ea = sm_pool.tile([128, T + 1], F32, tag="ea")
            nc.scalar.activation(ea[0:32, :], zg[0:32, 0:T + 1], AF.Exp,
                                 bias=negm[0:32, 0:1])
            ssum = sm_pool.tile([128, 1], F32, tag="ssum")
            nc.vector.tensor_reduce(ssum[0:32], ea[0:32, :], axis=AX.X,
                                    op=ALU.add)
            rinv = sm_pool.tile([128, 1], F32, tag="rinv")
            nc.vector.reciprocal(rinv[0:32], ssum[0:32])
            ab = sm_pool.tile([128, T + 1], BF16, tag="ab")
            nc.vector.tensor_scalar_mul(ab[0:32, :], ea[0:32, :],
                                        rinv[0:32, 0:1])

            # 8. transpose a -> aT columns (col = row-in-group bg; even bg
            # valid on partitions 0..49, odd bg on 64..113, zeros elsewhere)
            aT = aT_tiles[grp % 2]
            pa0 = c_ps.tile([128, 1024], BF16, tag="cps")
            nc.tensor.matmul(pa0[0:T + 1, 0:32], ab[0:32, :], identb[0:32, 0:32],
                             is_transpose=True, tile_position=(0, 0))
            nc.vector.tensor_copy(
                aT[0:T + 1, :].rearrange("p (c2 par) -> p c2 par", par=2)[:, :, 0],
                pa0[0:T + 1, 0:32].rearrange("p (c2 par) -> p c2 par", par=2)[:, :, 0])
            pa1 = c_ps.tile([128, 1024], BF16, tag="cps")
            nc.tensor.matmul(pa1[64:64 + T + 1, 0:32], ab[0:32, :],
                             identb[0:32, 0:32],
                             is_transpose=True, tile_position=(0, 64))
            nc.vector.tensor_copy(
                aT[64:64 + T + 1, :].rearrange(
                    "p (c2 par) -> p c2 par", par=2)[:, :, 1],
                pa1[64:64 + T + 1, 0:32].rearrange(
                    "p (c2 par) -> p c2 par", par=2)[:, :, 1])

            # 9. attention apply (op2): one M=2 matmul per slot (both
            # parities packed via zero-masked aT columns), then a single
            # full-tile bf16 drain per chunk and one store DMA per 2 chunks.
            for cj in range(4):
                ci = grp * 4 + cj
                b0 = ci * NB
                v2c = v2cs[cj]
                cps = c_ps.tile([128, H], F32, tag="cps", name="cps")
                for slot in range(4):
                    bg = cj * 8 + 2 * slot
                    nc.tensor.matmul(
                        cps[32 * slot:32 * slot + 2, :],
                        aT[:, bg:bg + 2],
                        v2c[:, slot, :],
                        start=True, stop=True,
                        tile_position=(0, 32 * slot))
                nc.vector.tensor_copy(cbf[:, cj, 0:H], cps[:])
            for slot in range(4):
                nc.scalar.dma_start(
                    cg[2 * slot:2 * slot + 2, 4 * grp:4 * grp + 4, :],
                    cbf[32 * slot:32 * slot + 2, :, 0:H])

    nc.compile()
    return nc


_NC_CACHE = {}

# test harness hooks: set TRACE=True (with an NTFF profile hook registered)
# to capture HW timing; the BassKernelResults of the last run lands in LAST.
TRACE = False
LAST = {}


def _get_nc(B):
    if B not in _NC_CACHE:
        _NC_CACHE[B] = build_bass(B)
    return _NC_CACHE[B]


def kernel(**inputs):
    from concourse.bass_utils import run_bass_kernel_spmd

    v = np.ascontiguousarray(np.asarray(inputs["v"], dtype=np.float32))
    h = np.ascontiguousarray(np.asarray(inputs["h"], dtype=np.float32))
    s = np.ascontiguousarray(np.asarray(inputs["s"], dtype=np.float32))
    B_total = v.shape[0]
    B = B_total // N_CORES
    nc = _get_nc(B)

    shared = {}
    for k in ("W_h", "b_h", "W_v", "b_v", "w_z", "W_s", "b_s", "w_beta"):
        shared[k] = np.ascontiguousarray(np.asarray(inputs[k], dtype=np.float32))
    for k in ("b_z", "b_beta"):
        shared[k] = np.asarray(inputs[k], dtype=np.float32).reshape(1)

    in_maps = []
    for k in range(N_CORES):
        sl = slice(k * B, (k + 1) * B)
        in_maps.append(dict(shared, v=v[sl], h=h[sl], s=s[sl]))

    kwargs = {"trace": True} if TRACE else {}
    res = run_bass_kernel_spmd(nc, in_maps, core_ids=list(range(N_CORES)),
                               **kwargs)
    LAST["res"] = res
    out = np.concatenate([r["c"] for r in res.results], axis=0)
    return out.astype(np.float32)
